# revision 18
# baseline (speedup 1.0000x reference)
"""Distributed Trainium2 kernel for GQA attention block (B=2,T=1024,D=2560,Nq=32,Nkv=8,H=128).

Sharding: 8 cores = 2 batches x 4 head-groups. Core c handles batch c//4 and
q-heads [8g:8g+8), kv-heads [2g:2g+2) where g=c%4. Attention is head-local;
two 8-core AllToAlls (even token chunks / odd token chunks) re-shard
heads->tokens before o_proj. Shard s of each A2A carries this core's heads for
token chunk 2*(s%4)(+1); each core keeps same-batch rows via a data-driven
select. Each core emits a [256, 2560] token-slice of the final output.
"""

import ml_dtypes
import numpy as np

import concourse.bass as bass
import concourse.mybir as mybir
import concourse.tile as tile
from concourse import bacc
from concourse.bass_utils import run_bass_kernel_spmd

F32 = mybir.dt.float32
F32R = mybir.dt.float32r
BF16 = mybir.dt.bfloat16

B, T, D, NQ, NKV, H = 2, 1024, 2560, 32, 8, 128
NDT = D // 128  # 20 contraction tiles
NTC = T // 128  # 8 token chunks
ROPE_THETA = 1000000.0
MROPE_SECTION = (24, 20, 20)
NORM_EPS = 1e-6
SOFT_SCALE = H ** -0.5
NEG = -1e30

EXP_ACT = mybir.ActivationFunctionType.Exp
SQUARE_ACT = mybir.ActivationFunctionType.Square
SQRT_ACT = mybir.ActivationFunctionType.Sqrt
MUL_OP = mybir.AluOpType.mult
ADD_OP = mybir.AluOpType.add

_LAST = None


def _norm_rope(nc, smpool, spool, ps, nh, eps_t, ct, s0t, s1t, tagp):
    """RMS-norm over h + rope for nh heads sitting in psum ps[:, :nh*128]."""
    w = nh * 128
    qn = spool.tile([128, w], F32, tag=f"qn{tagp}")
    ssq = smpool.tile([128, nh], F32, tag="ssq")
    for hh in range(nh):
        sq = smpool.tile([128, 128], F32, tag="sq")
        nc.scalar.activation(sq[:, :], ps[:, hh * 128:(hh + 1) * 128], SQUARE_ACT,
                             accum_out=ssq[:, hh:hh + 1])
    srt = smpool.tile([128, nh], F32, tag="srt")
    nc.scalar.activation(srt[:, :], ssq[:, :], SQRT_ACT, bias=eps_t[:, :], scale=1.0 / H)
    rsq = smpool.tile([128, nh], F32, tag="rsq")
    nc.vector.reciprocal(rsq[:, :], srt[:, :])
    for hh in range(nh):
        sl = slice(hh * 128, (hh + 1) * 128)
        nc.vector.tensor_scalar_mul(qn[:, sl], ps[:, sl], rsq[:, hh:hh + 1])
    qm = spool.tile([128, w], F32, tag=f"qm{tagp}")
    qr = spool.tile([128, w], F32, tag=f"qr{tagp}")
    cb = ct.unsqueeze(1).broadcast_to([128, nh, 128])
    s0b = s0t.unsqueeze(1).broadcast_to([128, nh, 64])
    s1b = s1t.unsqueeze(1).broadcast_to([128, nh, 64])
    qn4 = qn[:, :].rearrange("p (h x) -> p h x", h=nh)
    qm4 = qm[:, :].rearrange("p (h x) -> p h x", h=nh)
    qr4 = qr[:, :].rearrange("p (h x) -> p h x", h=nh)
    nc.vector.tensor_mul(qm4, qn4, cb)
    t1 = spool.tile([128, nh * 64], F32, tag=f"t1{tagp}")
    t14 = t1[:, :].rearrange("p (h x) -> p h x", h=nh)
    nc.vector.tensor_mul(t14, qn4[:, :, 64:128], s0b)
    nc.vector.tensor_sub(qr4[:, :, 0:64], qm4[:, :, 0:64], t14)
    t2 = spool.tile([128, nh * 64], F32, tag=f"t2{tagp}")
    t24 = t2[:, :].rearrange("p (h x) -> p h x", h=nh)
    nc.vector.tensor_mul(t24, qn4[:, :, 0:64], s1b)
    nc.vector.tensor_add(qr4[:, :, 64:128], qm4[:, :, 64:128], t24)
    return qr


def _build_nc():
    nc = bacc.Bacc(None, target_bir_lowering=False, num_devices=8)

    xt_e = nc.declare_dram_parameter("xt", [NTC, NDT, 128, 128], F32R, isOutput=False)
    wq0_e = nc.declare_dram_parameter("wq0", [NDT, 128, 512], F32R, isOutput=False)
    wq1_e = nc.declare_dram_parameter("wq1", [NDT, 128, 512], F32R, isOutput=False)
    wkv_e = nc.declare_dram_parameter("wkv", [NDT, 128, 512], F32R, isOutput=False)
    cq_e = nc.declare_dram_parameter("cq", [T, 128], F32, isOutput=False)
    s0q_e = nc.declare_dram_parameter("s0q", [T, 64], F32, isOutput=False)
    s1q_e = nc.declare_dram_parameter("s1q", [T, 64], F32, isOutput=False)
    ck_e = nc.declare_dram_parameter("ck", [T, 128], F32, isOutput=False)
    s0k_e = nc.declare_dram_parameter("s0k", [T, 64], F32, isOutput=False)
    s1k_e = nc.declare_dram_parameter("s1k", [T, 64], F32, isOutput=False)
    maska_e = nc.declare_dram_parameter("maska", [NTC, 128, 128], F32, isOutput=False)
    identf_e = nc.declare_dram_parameter("identf", [128, 128], F32, isOutput=False)
    identb_e = nc.declare_dram_parameter("identb", [128, 128], BF16, isOutput=False)
    wob_e = nc.declare_dram_parameter("wob", [128, NQ, D], BF16, isOutput=False)
    bsel_e = nc.declare_dram_parameter("bsel", [128, 1], F32, isOutput=False)
    bsm_e = nc.declare_dram_parameter("bsm", [128, 1], F32, isOutput=False)
    out_e = nc.declare_dram_parameter("out", [256, D], F32, isOutput=True)

    with tile.TileContext(nc) as tc:
        with (
            tc.tile_pool(name="const", bufs=1) as cpool,
            tc.tile_pool(name="dram", bufs=1, space="DRAM") as dpool,
        ):
            eps_t = cpool.tile([128, 1], F32, tag="eps")
            nc.gpsimd.memset(eps_t[:, :], NORM_EPS)
            identb = cpool.tile([128, 128], BF16, tag="identb")
            nc.sync.dma_start(out=identb[:, :], in_=identb_e[:, :])

            a2aA_in = dpool.tile([8, 8, 128, 128], BF16, tag="a2aA_in")
            a2aA_out = dpool.tile([8, 8, 128, 128], BF16, tag="a2aA_out")
            a2aB_in = dpool.tile([8, 8, 128, 128], BF16, tag="a2aB_in")
            a2aB_out = dpool.tile([8, 8, 128, 128], BF16, tag="a2aB_out")

            with tc.tile_pool(name="acts", bufs=1) as apool:
                qT = apool.tile([128, 8, T], F32R, tag="qT")       # [h, qhead, t]
                kT = apool.tile([128, 2, T], F32R, tag="kT")       # [h, kvhead, t]
                vN = apool.tile([128, NTC, 256], BF16, tag="vN")   # [t_in_chunk, chunk, kv*128+h]

                # ====== Phase 1 ======
                with (
                    tc.tile_pool(name="p1x", bufs=3) as xpool,
                    tc.tile_pool(name="p1w", bufs=21) as wpool,
                    tc.tile_pool(name="p1s", bufs=3) as spool,
                    tc.tile_pool(name="p1sm", bufs=8) as smpool,
                    tc.tile_pool(name="p1ps", bufs=2, space="PSUM") as pspool,
                    tc.tile_pool(name="p1pt", bufs=2, space="PSUM") as ptpool,
                ):
                    # first weight group + first x chunk go FIRST on the
                    # sync queue so the PE can start ASAP; tables after.
                    wts0 = []
                    for d in range(NDT):
                        wt = wpool.tile([128, 512], F32R, tag="wt")
                        nc.sync.dma_start(out=wt[:, :], in_=wq0_e[d, :, :])
                        wts0.append(wt)
                    xc0 = xpool.tile([128, NDT, 128], F32R, tag="xc")
                    nc.sync.dma_start(out=xc0[:, :, :],
                                      in_=xt_e[0, :, :, :].rearrange("d p t -> p d t"))

                    def ld(name, shp, src, rearr):
                        t = apool.tile(shp, F32, tag=name)
                        nc.sync.dma_start(out=t[:, :, :], in_=src[:, :].rearrange(rearr, p=128))
                        return t

                    cq = ld("cq", [128, NTC, 128], cq_e, "(c p) m -> p c m")
                    s0q = ld("s0q", [128, NTC, 64], s0q_e, "(c p) m -> p c m")
                    s1q = ld("s1q", [128, NTC, 64], s1q_e, "(c p) m -> p c m")
                    ck = ld("ck", [128, NTC, 128], ck_e, "(c p) m -> p c m")
                    s0k = ld("s0k", [128, NTC, 64], s0k_e, "(c p) m -> p c m")
                    s1k = ld("s1k", [128, NTC, 64], s1k_e, "(c p) m -> p c m")
                    maskt = apool.tile([128, NTC, 128], F32, tag="maskt")
                    nc.sync.dma_start(out=maskt[:, :, :], in_=maska_e[:, :, :].rearrange("i p m -> p i m"))
                    identf = apool.tile([128, 128], F32, tag="identf")
                    nc.sync.dma_start(out=identf[:, :], in_=identf_e[:, :])

                    # deferred PE-transposes: run one iteration behind the
                    # matmuls so the PE never waits on the DVE rope chain
                    pend1 = []

                    def flush1():
                        for qr_, tch_, heads_, dest in pend1:
                            for idx, head in enumerate(heads_):
                                pt = ptpool.tile([128, 128], F32, tag="pt")
                                nc.tensor.transpose(pt[:, :], qr_[:, idx * 128:(idx + 1) * 128], identf[:, :])
                                nc.vector.tensor_copy(dest[:, head, tch_ * 128:(tch_ + 1) * 128], pt[:, :])
                        pend1.clear()

                    for grp in range(3):
                        if grp == 0:
                            wts = wts0
                        else:
                            wdram = [wq0_e, wq1_e, wkv_e][grp]
                            wts = []
                            for d in range(NDT):
                                wt = wpool.tile([128, 512], F32R, tag="wt")
                                nc.sync.dma_start(out=wt[:, :], in_=wdram[d, :, :])
                                wts.append(wt)
                        for tch in range(NTC):
                            if grp == 0 and tch == 0:
                                xc = xc0
                            else:
                                xc = xpool.tile([128, NDT, 128], F32R, tag="xc")
                                nc.sync.dma_start(
                                    out=xc[:, :, :],
                                    in_=xt_e[tch, :, :, :].rearrange("d p t -> p d t"),
                                )
                            ps = pspool.tile([128, 512], F32, tag="ps")
                            for d in range(NDT):
                                nc.tensor.matmul(
                                    ps[:, :], xc[:, d, :], wts[d][:, :],
                                    start=(d == 0), stop=(d == NDT - 1),
                                )
                            flush1()
                            if grp < 2:
                                qr = _norm_rope(
                                    nc, smpool, spool, ps[:, :], 4, eps_t,
                                    cq[:, tch, :], s0q[:, tch, :], s1q[:, tch, :], "q")
                                pend1.append((qr, tch, [grp * 4 + hh for hh in range(4)], qT))
                            else:
                                kr = _norm_rope(
                                    nc, smpool, spool, ps[:, 0:256], 2, eps_t,
                                    ck[:, tch, :], s0k[:, tch, :], s1k[:, tch, :], "k")
                                pend1.append((kr, tch, [0, 1], kT))
                                nc.vector.tensor_copy(vN[:, tch, :], ps[:, 256:512])
                    flush1()

                # ====== Phase 2: attention (software-pipelined, evens then odds) ======
                with (
                    tc.tile_pool(name="p2a", bufs=3) as aapool,
                    tc.tile_pool(name="p2t", bufs=4) as tpool,
                    tc.tile_pool(name="p2d", bufs=3) as dpool2,
                    tc.tile_pool(name="p2o", bufs=2) as opool,
                    tc.tile_pool(name="p2sm", bufs=6) as sm2pool,
                    tc.tile_pool(name="p2sc", bufs=2, space="PSUM") as scpool,
                    tc.tile_pool(name="p2tr", bufs=2, space="PSUM") as trpool,
                    tc.tile_pool(name="p2av", bufs=2, space="PSUM") as avpool,
                ):
                    def finish(st):
                        hq, i, kv, at, drcp, oti = st
                        ov = avpool.tile([128, 128], F32, tag="ov")
                        for j in range(i + 1):
                            pt = trpool.tile([128, 128], F32, tag="ptr")
                            # transpose + softmax normalization in one PE op:
                            # regular matmul at_block.T @ diag(1/rowsum)
                            nc.tensor.matmul(pt[:, :], at[:, j * 128:(j + 1) * 128], drcp[:, :],
                                             start=True, stop=True)
                            atj = tpool.tile([128, 128], BF16, tag="atj")
                            nc.vector.tensor_copy(atj[:, :], pt[:, :])
                            nc.tensor.matmul(
                                ov[:, :],
                                vN[:, j, kv * 128:(kv + 1) * 128],
                                atj[:, :],
                                start=(j == 0),
                                stop=(j == i),
                            )
                        nc.vector.tensor_copy(oti[:, hq, :], ov[:, :])
                        if hq == 7:
                            r = i // 2
                            dst = a2aA_in if i % 2 == 0 else a2aB_in
                            nc.sync.dma_start(
                                out=dst[r, :, :, :].rearrange("l p t -> p l t"),
                                in_=oti[:, :, :],
                            )
                            nc.sync.dma_start(
                                out=dst[4 + r, :, :, :].rearrange("l p t -> p l t"),
                                in_=oti[:, :, :],
                            )

                    prev = None
                    for i in [0, 2, 4, 6, 1, 3, 5, 7]:
                        klen = 128 * (i + 1)
                        oti = opool.tile([128, 8, 128], BF16, tag="oti")
                        for hq in range(8):
                            kv = hq // 4
                            sc = scpool.tile([128, T], F32, tag="sc")
                            lhsT = qT[:, hq, i * 128:(i + 1) * 128]
                            for j0 in range(0, klen, 512):
                                j1 = min(klen, j0 + 512)
                                nc.tensor.matmul(
                                    sc[:, j0:j1], lhsT, kT[:, kv, j0:j1],
                                    start=True, stop=True,
                                )
                            nc.vector.tensor_add(sc[:, klen - 128:klen], sc[:, klen - 128:klen], maskt[:, i, :])
                            at = aapool.tile([128, T], BF16, tag="at")
                            rs = sm2pool.tile([128, 1], F32, tag="rs")
                            nc.scalar.activation(at[:, :klen], sc[:, :klen], EXP_ACT, scale=SOFT_SCALE, accum_out=rs[:, :])
                            rcp = sm2pool.tile([128, 1], F32, tag="rcp")
                            nc.vector.reciprocal(rcp[:, :], rs[:, :])
                            drcp = dpool2.tile([128, 128], BF16, tag="drcp")
                            nc.vector.tensor_scalar_mul(drcp[:, :], identb[:, :], rcp[:, :])
                            cur = (hq, i, kv, at, drcp, oti)
                            if prev is not None:
                                finish(prev)
                            prev = cur
                        if i == 6:
                            finish(prev)
                            prev = None
                            nc.gpsimd.collective_compute(
                                "AllToAll",
                                mybir.AluOpType.bypass,
                                replica_groups=[[0, 1, 2, 3, 4, 5, 6, 7]],
                                ins=[a2aA_in[:, :, :, :].opt()],
                                outs=[a2aA_out[:, :, :, :].opt()],
                            )
                    finish(prev)

            # ====== Phase 3: AllToAll(B) + select + o_proj ======
            with (
                tc.tile_pool(name="p3a", bufs=1) as a3pool,
                tc.tile_pool(name="p3t", bufs=2) as t3pool,
                tc.tile_pool(name="p3w", bufs=3) as w3pool,
                tc.tile_pool(name="p3o", bufs=3) as o3pool,
                tc.tile_pool(name="p3ps", bufs=2, space="PSUM") as ps3pool,
            ):
                # wo prefetch + bsel first on the sync queue (independent of
                # the collectives)
                bsel = a3pool.tile([128, 1], F32, tag="bsel")
                nc.sync.dma_start(out=bsel[:, :], in_=bsel_e[:, :])
                bsm = a3pool.tile([128, 1], F32, tag="bsm")
                nc.sync.dma_start(out=bsm[:, :], in_=bsm_e[:, :])
                wo_ts = []
                for dch in range(5):
                    wo_t = w3pool.tile([128, NQ, 512], BF16, tag="wo_t")
                    nc.sync.dma_start(out=wo_t[:, :, :], in_=wob_e[:, :, dch * 512:(dch + 1) * 512])
                    wo_ts.append(wo_t)

                nc.gpsimd.collective_compute(
                    "AllToAll",
                    mybir.AluOpType.bypass,
                    replica_groups=[[0, 1, 2, 3, 4, 5, 6, 7]],
                    ins=[a2aB_in[:, :, :, :].opt()],
                    outs=[a2aB_out[:, :, :, :].opt()],
                )

                def selects(par, a2a_out_t):
                    res = []
                    for g4 in range(4):
                        lo = a3pool.tile([128, 8, 128], BF16, tag=f"lo{par}{g4}")
                        nc.gpsimd.dma_start(
                            out=lo[:, :, :],
                            in_=a2a_out_t[g4, :, :, :].rearrange("l h t -> h l t"),
                        )
                        hi = a3pool.tile([128, 8, 128], BF16, tag=f"hi{par}{g4}")
                        nc.gpsimd.dma_start(
                            out=hi[:, :, :],
                            in_=a2a_out_t[4 + g4, :, :, :].rearrange("l h t -> h l t"),
                        )
                        tt = t3pool.tile([128, 8, 128], BF16, tag="tt")
                        nc.vector.tensor_scalar_mul(tt[:, :, :], lo[:, :, :], bsel[:, :])
                        aT = a3pool.tile([128, 8, 128], BF16, tag=f"aT{par}{g4}")
                        nc.vector.scalar_tensor_tensor(
                            aT[:, :, :], hi[:, :, :], bsm[:, :], tt[:, :, :],
                            op0=MUL_OP, op1=ADD_OP,
                        )
                        res.append(aT)
                    return res

                def oproj(tq, aTs, dchs, wts3):
                    for dch in dchs:
                        po = ps3pool.tile([128, 512], F32, tag="po")
                        for n in range(NQ):
                            nc.tensor.matmul(
                                po[:, :],
                                aTs[n // 8][:, n % 8, :],
                                wts3[dch][:, n, :],
                                start=(n == 0),
                                stop=(n == NQ - 1),
                            )
                        ob = o3pool.tile([128, 512], F32, tag="ob")
                        nc.vector.tensor_copy(ob[:, :], po[:, :])
                        nc.sync.dma_start(
                            out=out_e[tq * 128:(tq + 1) * 128, dch * 512:(dch + 1) * 512],
                            in_=ob[:, :],
                        )

                aTe = selects(0, a2aA_out)
                # tq=0 (even chunks) depends only on A2A#A -> runs while #B flies
                oproj(0, aTe, [0, 1, 2, 3, 4], wo_ts)
                aTo = selects(1, a2aB_out)
                # reuse the wo tiles still resident (slots 2,3,4); re-load 0,1
                wo_b = {2: wo_ts[2], 3: wo_ts[3], 4: wo_ts[4]}
                for dch in (1, 0):
                    wo_t = w3pool.tile([128, NQ, 512], BF16, tag="wo_t")
                    nc.sync.dma_start(out=wo_t[:, :, :], in_=wob_e[:, :, dch * 512:(dch + 1) * 512])
                    wo_b[dch] = wo_t
                oproj(1, aTo, [4, 3, 2, 1, 0], wo_b)
    return nc


def _rope_tables(pos_b):
    """pos_b: [3, T] int32 -> sin/cos [T, 64] per mrope."""
    fraction = 2.0 * np.arange(0, H // 2, dtype=np.float64) / H
    timescale = ROPE_THETA ** fraction
    sinusoid = pos_b[:, :, None].astype(np.float64) / timescale  # [3, T, 64]
    freq = sinusoid[0].copy()
    h_idx = np.arange(1, MROPE_SECTION[1] * 3, 3)
    w_idx = np.arange(2, MROPE_SECTION[2] * 3, 3)
    freq[:, h_idx] = sinusoid[1][:, h_idx]
    freq[:, w_idx] = sinusoid[2][:, w_idx]
    return np.sin(freq).astype(np.float32), np.cos(freq).astype(np.float32)


def _tables_for(sin, cos, w):
    """C [T,128], S0 [T,64], S1 [T,64] with norm weight w [128] folded in."""
    C = np.concatenate([cos * w[None, :64], cos * w[None, 64:]], axis=1)
    S0 = sin * w[None, 64:]
    S1 = sin * w[None, :64]
    return (np.ascontiguousarray(C), np.ascontiguousarray(S0), np.ascontiguousarray(S1))


def kernel(x, positions, attn_mask, wq, wk, wv, wo, q_norm_w, k_norm_w):
    x = np.asarray(x, dtype=np.float32)
    positions = np.asarray(positions)
    attn_mask = np.asarray(attn_mask)
    wq = np.asarray(wq, dtype=np.float32)
    wk = np.asarray(wk, dtype=np.float32)
    wv = np.asarray(wv, dtype=np.float32)
    wo = np.asarray(wo, dtype=np.float32)
    q_norm_w = np.asarray(q_norm_w, dtype=np.float32)
    k_norm_w = np.asarray(k_norm_w, dtype=np.float32)

    nc = _build_nc()
    nc.finalize()

    identf = np.eye(128, dtype=np.float32)
    identb = np.eye(128).astype(ml_dtypes.bfloat16)
    wob = np.ascontiguousarray(wo.transpose(1, 0, 2)).astype(ml_dtypes.bfloat16)

    in_maps = []
    for c in range(8):
        b, g = c // 4, c % 4
        xt = np.ascontiguousarray(
            x[b].T.reshape(NDT, 128, NTC, 128).transpose(2, 0, 1, 3))
        kvh = slice(g * 2, g * 2 + 2)
        wq0 = np.ascontiguousarray(
            wq[:, g * 8:g * 8 + 4, :].reshape(D, 512).reshape(NDT, 128, 512))
        wq1 = np.ascontiguousarray(
            wq[:, g * 8 + 4:g * 8 + 8, :].reshape(D, 512).reshape(NDT, 128, 512))
        wkv = np.ascontiguousarray(
            np.concatenate([
                wk[:, kvh, :].reshape(D, 256),
                wv[:, kvh, :].reshape(D, 256),
            ], axis=1).reshape(NDT, 128, 512))
        sin, cos = _rope_tables(np.asarray(positions[:, b, :]))
        cqt, s0qt, s1qt = _tables_for(sin, cos, q_norm_w)
        ckt, s0kt, s1kt = _tables_for(sin, cos, k_norm_w)
        mb = np.empty((NTC, 128, 128), np.float32)
        for i in range(NTC):
            blk = attn_mask[b, i * 128:(i + 1) * 128, i * 128:(i + 1) * 128]
            mb[i] = np.where(blk, 0.0, NEG)
        in_maps.append({
            "xt": xt, "wq0": wq0, "wq1": wq1, "wkv": wkv,
            "cq": cqt, "s0q": s0qt, "s1q": s1qt,
            "ck": ckt, "s0k": s0kt, "s1k": s1kt,
            "maska": mb, "identf": identf, "identb": identb, "wob": wob,
            "bsel": np.full((128, 1), 1.0 if b == 0 else 0.0, np.float32),
            "bsm": np.full((128, 1), 0.0 if b == 0 else 1.0, np.float32),
        })

    res = run_bass_kernel_spmd(nc, in_maps, core_ids=list(range(8)))
    global _LAST
    _LAST = res
    full = np.empty((B, T, D), np.float32)
    for c in range(8):
        b, g = c // 4, c % 4
        full[b, g * 256:(g + 1) * 256, :] = res.results[c]["out"]
    return full
